# revision 17
# baseline (speedup 1.0000x reference)
"""DenseGraphAttentionHead Trainium2 Bass kernel (8-core SPMD row-sharded), v3.

reference math:
    Wh = nodes @ W_w.T + W_b                    [N, 256]
    Wh1 = Wh @ a1_w.T + a1_b                    [N, 1]
    Wh2 = Wh @ a2_w.T + a2_b                    [N, 1]
    scores = leaky_relu(Wh1 + Wh2.T, 0.2)       [N, N]
    attention = softmax(where(edge, scores, -inf), axis=1)
    out = attention @ Wh                        [N, 256]

Key identity: softmax over j is invariant to per-row(i) factors, so with
    p[i] = exp(0.8*Wh1[i]),  q[j] = exp(0.2*Wh2[j]),  rq[j] = exp(0.8*Wh2[j])
we have  attention_ij ∝ edge_ij * q[j] * max(1, rq[j]*p[i]).
Folding q[j] into Wh_aug (scale applied during the PSUM->SBUF copy on the
scalar engine) leaves the per-chunk score op as a two-scalar tensor_scalar
with an immediate second operand: w = max(1, rq*p), then Y = w*mask.

Per core c (rows i in [c*1024, (c+1)*1024), scores in [j(part), i(free)]):
  - Phase 1 (build): pwh[j,0:258] = nodes @ [W.T | 0 | v2] (fp16 matmuls);
    wh_aug[:, ck, 0:256] = q_j * Wh (ACT copy, scale=q ptr); col 257 of psum
    = Wh2 feeds the q/rq exps; wh_aug[:, :, 256] = q_j (denominator column).
  - Phase 2 (sweep): full-width i (1024); per chunk TS w=max(1, rq*p) (DVE),
    per group TT Y = w (.) mask [128, 4, 1024] (DVE).  8 PSUM accumulators
    accumulate Y.T @ Wh_aug over 64 j-chunks; col 256 = softmax denominator.
  - out = acc[:, :256]/acc[:, 256] + W_b.
Mask staged host-side as [16, 128, 4096] fp8 so each DMA partition line is
4KB contiguous; upcast to fp16 during the SWDGE DMA.
"""
import sys
import types

import numpy as np

N_NODES = 8192
IN_DIM = 512
OUT_DIM = 256
ALPHA = 0.2
N_CORES = 8
ROWS = N_NODES // N_CORES          # 1024 rows per core
NCK = N_NODES // 128               # 64 j-chunks of 128
GRP = 4                            # j-chunks per mask-DMA group
NG = NCK // GRP                    # 16 groups

_CACHE = {}


def _ensure_ntff_hook():
    """antenv.axon_hooks is absent in this container; shim it so
    run_bass_kernel_spmd(trace=True) can reach the NTFF profiler."""
    if "antenv.axon_hooks" in sys.modules:
        return
    holder = [None]
    mod = types.ModuleType("antenv.axon_hooks")
    mod.set_axon_ntff_profile_hook = lambda h: holder.__setitem__(0, h)
    mod.get_axon_ntff_profile_hook = lambda: holder[0]
    sys.modules["antenv.axon_hooks"] = mod
    try:
        from trn_agent_boot.trn_boot import _ntff_profile_via_ctypes
        mod.set_axon_ntff_profile_hook(
            _ntff_profile_via_ctypes("/opt/axon/libaxon_pjrt.so"))
    except Exception:
        pass


def _build_nc():
    import concourse.bacc as bacc
    import concourse.tile as tile
    from concourse import mybir

    F16 = mybir.dt.float16
    F32 = mybir.dt.float32
    F8 = mybir.dt.float8e4
    ADD = mybir.AluOpType.add
    MULT = mybir.AluOpType.mult
    MAX = mybir.AluOpType.max
    EXP = mybir.ActivationFunctionType.Exp

    nc = bacc.Bacc("TRN2", target_bir_lowering=False, debug=False,
                   num_devices=N_CORES)

    nodesT_d = nc.dram_tensor("nodesT", [IN_DIM, N_NODES], F16,
                              kind="ExternalInput")
    ndown_d = nc.dram_tensor("nodesT_own", [IN_DIM, ROWS], F16,
                             kind="ExternalInput")
    maskm_d = nc.dram_tensor("maskm", [NG, 128, GRP * ROWS], F8,
                             kind="ExternalInput")
    wtaug_d = nc.dram_tensor("wt_aug", [IN_DIM, 258], F16,
                             kind="ExternalInput")
    v1_d = nc.dram_tensor("v1", [IN_DIM, 1], F16, kind="ExternalInput")
    c1_d = nc.dram_tensor("c1", [1, 1], F32, kind="ExternalInput")
    c2_d = nc.dram_tensor("c2qr", [128, 2], F32, kind="ExternalInput")
    # raw numerator|denominator; the divide + W_b add happen host-side
    out_d = nc.dram_tensor("out", [ROWS, 257], F32, kind="ExternalOutput")

    with tile.TileContext(nc) as tc:
        with (
            tc.tile_pool(name="consts", bufs=1) as consts,
            tc.tile_pool(name="ndpool", bufs=2) as ndpool,
            tc.tile_pool(name="mgp", bufs=3) as mgp,
            tc.tile_pool(name="sgp", bufs=8) as sgp,
            tc.tile_pool(name="xgp", bufs=3) as xgp,
            tc.tile_pool(name="outp", bufs=2) as outp,
        ):
            # ---- constants (wt first so the build can start ASAP) ----
            wt_t = []
            v1_t = []
            ndown_t = []
            for d4 in range(4):
                w = consts.tile([128, 258], F16, name=f"wt{d4}", tag=f"wt{d4}")
                nc.sync.dma_start(w[:], wtaug_d[d4 * 128:(d4 + 1) * 128, :])
                wt_t.append(w)
            for d4 in range(4):
                v = consts.tile([128, 1], F16, name=f"v1_{d4}", tag=f"v1_{d4}")
                nc.sync.dma_start(v[:], v1_d[d4 * 128:(d4 + 1) * 128, :])
                v1_t.append(v[:])
                nd = consts.tile([128, ROWS], F16, name=f"ndo{d4}",
                                 tag=f"ndo{d4}")
                nc.sync.dma_start(nd[:], ndown_d[d4 * 128:(d4 + 1) * 128, :])
                ndown_t.append(nd[:])
            c1 = consts.tile([1, 1], F32)
            nc.sync.dma_start(c1[:], c1_d[:])
            c2qr = consts.tile([128, 2], F32)
            nc.sync.dma_start(c2qr[:], c2_d[:])

            wh_aug = consts.tile([128, NCK, 257], F16)
            wh2raw = consts.tile([128, NCK], F32)
            q128 = consts.tile([128, NCK], F32)
            r128 = consts.tile([128, NCK], F32)

            # ---- phase 1: Wh1 row -> p, and the q-scaled Wh_aug build ----
            with tc.tile_pool(name="psB", bufs=1, space="PSUM") as psB:
                wh1row = consts.tile([1, ROWS], F16)
                for h2 in range(2):
                    pw1 = psB.tile([1, 512], F32, name="pw1", tag="pw1",
                                   bufs=2)
                    for d4 in range(4):
                        nc.tensor.matmul(
                            pw1[:], v1_t[d4],
                            ndown_t[d4][:, h2 * 512:(h2 + 1) * 512],
                            start=(d4 == 0), stop=(d4 == 3),
                            skip_group_check=True)
                    nc.vector.tensor_scalar(
                        wh1row[:, h2 * 512:(h2 + 1) * 512], pw1[:], c1[:],
                        None, op0=ADD)
                p_row = consts.tile([1, ROWS], F16)
                nc.scalar.activation(p_row[:], wh1row[:], EXP, scale=ALPHA * 4)
                p_b = consts.tile([128, ROWS], F16)
                nc.gpsimd.partition_broadcast(p_b[:], p_row[:])

                for b in range(8):
                    ndT = ndpool.tile([128, 4, ROWS], F16, name="ndT",
                                      tag="ndT")
                    for d4 in range(4):
                        nc.sync.dma_start(
                            ndT[:, d4, :],
                            nodesT_d[d4 * 128:(d4 + 1) * 128,
                                     b * 1024:(b + 1) * 1024])
                    for ckl in range(8):
                        ck = b * 8 + ckl
                        pwh = psB.tile([128, 258], F32, name="pwh", tag="pwh",
                                       bufs=3)
                        for d4 in range(4):
                            nc.tensor.matmul(
                                pwh[:],
                                ndT[:, d4, ckl * 128:(ckl + 1) * 128],
                                wt_t[d4][:],
                                start=(d4 == 0), stop=(d4 == 3),
                                skip_group_check=True)
                        # extract Wh2 (feeds q/r exps), then plain copy
                        nc.scalar.copy(wh2raw[:, ck:ck + 1], pwh[:, 257:258])
                        if ckl == 7:
                            sl = slice(b * 8, (b + 1) * 8)
                            nc.scalar.activation(q128[:, sl], wh2raw[:, sl],
                                                 EXP, scale=ALPHA,
                                                 bias=c2qr[:, 0:1])
                            nc.scalar.activation(r128[:, sl], wh2raw[:, sl],
                                                 EXP, scale=1.0,
                                                 bias=c2qr[:, 1:2])
                        nc.scalar.copy(wh_aug[:, ck, 0:256], pwh[:, 0:256])
                nc.gpsimd.memset(wh_aug[:, :, 256:257], 1.0)

            # ---- phase 2: masked-score sweep, 8 full-PSUM accumulators ----
            with tc.tile_pool(name="psA", bufs=1, space="PSUM") as psA:
                accs = [psA.tile([128, 512], F32, name=f"acc{ib}",
                                 tag=f"acc{ib}") for ib in range(8)]
                for g in range(NG):
                    mgrp = mgp.tile([128, GRP, ROWS], F16, name="mgrp",
                                    tag="mgrp")
                    nc.gpsimd.dma_start(mgrp[:], maskm_d[g],
                                        max_dma_last_dim=512)
                    wgrp = sgp.tile([128, GRP, ROWS], F16, name="wgrp",
                                    tag="wgrp", bufs=2)
                    for ckl in range(GRP):
                        ck = g * GRP + ckl
                        nc.vector.tensor_scalar(
                            wgrp[:, ckl, :], p_b[:], r128[:, ck:ck + 1],
                            q128[:, ck:ck + 1], op0=MULT, op1=MAX)
                    ygrp = xgp.tile([128, GRP, ROWS], F16, name="ygrp",
                                    tag="ygrp")
                    nc.vector.tensor_tensor(ygrp[:], wgrp[:], mgrp[:],
                                            op=MULT)
                    for ckl in range(GRP):
                        ck = g * GRP + ckl
                        for ib in range(8):
                            nc.tensor.matmul(
                                accs[ib][:, 0:257],
                                ygrp[:, ckl, ib * 128:(ib + 1) * 128],
                                wh_aug[:, ck, :],
                                start=(ck == 0), stop=(ck == NCK - 1),
                                skip_group_check=True)
                for ib in range(8):
                    o = outp.tile([128, 257], F32, name="o", tag="o")
                    nc.scalar.copy(o[:], accs[ib][:, 0:257])
                    r0 = ib * 128
                    nc.sync.dma_start(out_d[r0:r0 + 128, :], o[:])
    nc.compile()
    return nc


def _get_nc():
    if "nc" not in _CACHE:
        _CACHE["nc"] = _build_nc()
    return _CACHE["nc"]


def _prep_in_maps(nodes, edge_mat, W_w, W_b, a1_w, a1_b, a2_w, a2_b):
    import ml_dtypes
    f16 = np.float16
    f8 = ml_dtypes.float8_e4m3fn
    nodes = np.asarray(nodes, dtype=np.float32)
    edge_mat = np.asarray(edge_mat, dtype=bool)
    W_w = np.asarray(W_w, dtype=np.float32)
    W_b = np.asarray(W_b, dtype=np.float32)
    a1_w = np.asarray(a1_w, dtype=np.float32)
    a1_b = np.asarray(a1_b, dtype=np.float32)
    a2_w = np.asarray(a2_w, dtype=np.float32)
    a2_b = np.asarray(a2_b, dtype=np.float32)

    nodesT = np.ascontiguousarray(nodes.T).astype(f16)          # [512, 8192]
    v1 = (W_w.T @ a1_w[0]).astype(f16)[:, None]                 # [512, 1]
    v2 = (W_w.T @ a2_w[0]).astype(f16)[:, None]
    wt_aug = np.concatenate(
        [W_w.T.astype(f16), np.zeros((IN_DIM, 1), f16), v2], axis=1)
    c1v = float(W_b @ a1_w[0]) + float(a1_b[0])
    c2v = float(W_b @ a2_w[0]) + float(a2_b[0])
    c1 = np.array([[c1v]], np.float32)
    # q = exp(ALPHA*(wh2 + c2)); r = exp(wh2 + c2)
    c2qr = np.broadcast_to(
        np.array([ALPHA * c2v, c2v], np.float32)[None, :], (128, 2)).copy()

    # mask {0,1} fp8 grouped: m[g, p, ckl*1024 + i] = edge[i_glob, (g*4+ckl)*128+p]
    em8 = np.where(edge_mat, 1, 0).astype(f8)

    in_maps = []
    for c in range(N_CORES):
        sl = slice(c * ROWS, (c + 1) * ROWS)
        sub = np.ascontiguousarray(em8[sl, :].T)               # [8192 j, 1024 i]
        maskm = np.ascontiguousarray(
            sub.reshape(NG, GRP, 128, ROWS).transpose(0, 2, 1, 3)
        ).reshape(NG, 128, GRP * ROWS)
        in_maps.append({
            "nodesT": nodesT,
            "nodesT_own": np.ascontiguousarray(nodesT[:, sl]),
            "maskm": maskm,
            "wt_aug": wt_aug,
            "v1": v1,
            "c1": c1,
            "c2qr": c2qr,
        })
    return in_maps


def _run(inputs, trace=False, trace_cores=None):
    from concourse.bass_utils import run_bass_kernel_spmd
    if trace:
        _ensure_ntff_hook()
    nc = _get_nc()
    in_maps = _prep_in_maps(**inputs)
    res = run_bass_kernel_spmd(nc, in_maps, list(range(N_CORES)),
                               trace=trace, trace_cores=trace_cores)
    raw = np.concatenate([res.results[c]["out"] for c in range(N_CORES)],
                         axis=0)                               # [N, 257]
    W_b = np.asarray(inputs["W_b"], dtype=np.float32)
    out = raw[:, :OUT_DIM] / raw[:, OUT_DIM:OUT_DIM + 1] + W_b[None, :]
    return out.astype(np.float32), res


def kernel(**inputs) -> np.ndarray:
    out, _ = _run(inputs, trace=False)
    return out


# revision 20
# speedup vs baseline: 1.1073x; 1.1073x over previous
"""DenseGraphAttentionHead Trainium2 Bass kernel (8-core SPMD row-sharded), v3.

reference math:
    Wh = nodes @ W_w.T + W_b                    [N, 256]
    Wh1 = Wh @ a1_w.T + a1_b                    [N, 1]
    Wh2 = Wh @ a2_w.T + a2_b                    [N, 1]
    scores = leaky_relu(Wh1 + Wh2.T, 0.2)       [N, N]
    attention = softmax(where(edge, scores, -inf), axis=1)
    out = attention @ Wh                        [N, 256]

Key identity: softmax over j is invariant to per-row(i) factors, so with
    p[i] = exp(0.8*Wh1[i]),  q[j] = exp(0.2*Wh2[j]),  rq[j] = exp(0.8*Wh2[j])
we have  attention_ij ∝ edge_ij * q[j] * max(1, rq[j]*p[i]).
Folding q[j] into Wh_aug (scale applied during the PSUM->SBUF copy on the
scalar engine) leaves the per-chunk score op as a two-scalar tensor_scalar
with an immediate second operand: w = max(1, rq*p), then Y = w*mask.

Per core c (rows i in [c*1024, (c+1)*1024), scores in [j(part), i(free)]):
  - Phase 1 (build): pwh[j,0:258] = nodes @ [W.T | 0 | v2] (fp16 matmuls);
    wh_aug[:, ck, 0:256] = q_j * Wh (ACT copy, scale=q ptr); col 257 of psum
    = Wh2 feeds the q/rq exps; wh_aug[:, :, 256] = q_j (denominator column).
  - Phase 2 (sweep): full-width i (1024); per chunk TS w=max(1, rq*p) (DVE),
    per group TT Y = w (.) mask [128, 4, 1024] (DVE).  8 PSUM accumulators
    accumulate Y.T @ Wh_aug over 64 j-chunks; col 256 = softmax denominator.
  - out = acc[:, :256]/acc[:, 256] + W_b.
Mask staged host-side as [16, 128, 4096] fp8 so each DMA partition line is
4KB contiguous; upcast to fp16 during the SWDGE DMA.
"""
import sys
import types

import numpy as np

N_NODES = 8192
IN_DIM = 512
OUT_DIM = 256
ALPHA = 0.2
N_CORES = 8
ROWS = N_NODES // N_CORES          # 1024 rows per core
NCK = N_NODES // 128               # 64 j-chunks of 128
GRP = 4                            # j-chunks per mask-DMA group
NG = NCK // GRP                    # 16 groups

_CACHE = {}


def _ensure_ntff_hook():
    """antenv.axon_hooks is absent in this container; shim it so
    run_bass_kernel_spmd(trace=True) can reach the NTFF profiler."""
    if "antenv.axon_hooks" in sys.modules:
        return
    holder = [None]
    mod = types.ModuleType("antenv.axon_hooks")
    mod.set_axon_ntff_profile_hook = lambda h: holder.__setitem__(0, h)
    mod.get_axon_ntff_profile_hook = lambda: holder[0]
    sys.modules["antenv.axon_hooks"] = mod
    try:
        from trn_agent_boot.trn_boot import _ntff_profile_via_ctypes
        mod.set_axon_ntff_profile_hook(
            _ntff_profile_via_ctypes("/opt/axon/libaxon_pjrt.so"))
    except Exception:
        pass


def _build_nc():
    import concourse.bacc as bacc
    import concourse.tile as tile
    from concourse import mybir

    F16 = mybir.dt.float16
    F32 = mybir.dt.float32
    F8 = mybir.dt.float8e4
    ADD = mybir.AluOpType.add
    MULT = mybir.AluOpType.mult
    MAX = mybir.AluOpType.max
    EXP = mybir.ActivationFunctionType.Exp

    nc = bacc.Bacc("TRN2", target_bir_lowering=False, debug=False,
                   num_devices=N_CORES)

    nodesT_d = nc.dram_tensor("nodesT", [IN_DIM, N_NODES], F16,
                              kind="ExternalInput")
    ndown_d = nc.dram_tensor("nodesT_own", [IN_DIM, ROWS], F16,
                             kind="ExternalInput")
    maskm_d = nc.dram_tensor("maskm", [NG, 128, GRP * ROWS], F8,
                             kind="ExternalInput")
    wtaug_d = nc.dram_tensor("wt_aug", [IN_DIM, 258], F16,
                             kind="ExternalInput")
    v1_d = nc.dram_tensor("v1", [IN_DIM, 1], F16, kind="ExternalInput")
    c1_d = nc.dram_tensor("c1", [1, 1], F32, kind="ExternalInput")
    c2_d = nc.dram_tensor("c2qr", [128, 2], F32, kind="ExternalInput")
    # raw numerator|denominator; the divide + W_b add happen host-side
    out_d = nc.dram_tensor("out", [ROWS, 257], F32, kind="ExternalOutput")

    with tile.TileContext(nc) as tc:
        with (
            tc.tile_pool(name="consts", bufs=1) as consts,
            tc.tile_pool(name="ndpool", bufs=2) as ndpool,
            tc.tile_pool(name="mgp", bufs=3) as mgp,
            tc.tile_pool(name="sgp", bufs=8) as sgp,
            tc.tile_pool(name="xgp", bufs=3) as xgp,
            tc.tile_pool(name="outp", bufs=2) as outp,
        ):
            # ---- constants (wt first so the build can start ASAP) ----
            wt_t = []
            v1_t = []
            ndown_t = []
            for d4 in range(4):
                w = consts.tile([128, 258], F16, name=f"wt{d4}", tag=f"wt{d4}")
                nc.sync.dma_start(w[:], wtaug_d[d4 * 128:(d4 + 1) * 128, :])
                wt_t.append(w)
            for d4 in range(4):
                v = consts.tile([128, 1], F16, name=f"v1_{d4}", tag=f"v1_{d4}")
                nc.sync.dma_start(v[:], v1_d[d4 * 128:(d4 + 1) * 128, :])
                v1_t.append(v[:])
                nd = consts.tile([128, ROWS], F16, name=f"ndo{d4}",
                                 tag=f"ndo{d4}")
                nc.sync.dma_start(nd[:], ndown_d[d4 * 128:(d4 + 1) * 128, :])
                ndown_t.append(nd[:])
            c1 = consts.tile([1, 1], F32)
            nc.sync.dma_start(c1[:], c1_d[:])
            c2qr = consts.tile([128, 2], F32)
            nc.sync.dma_start(c2qr[:], c2_d[:])

            # cols 0:256 = Wh, col 256 = 1.0 (denominator), col 257 = Wh2
            wh_aug = consts.tile([128, NCK, 258], F16)
            q128 = consts.tile([128, NCK], F32)
            r128 = consts.tile([128, NCK], F32)

            # ---- phase 1: Wh1 row -> p, and the q-scaled Wh_aug build ----
            with tc.tile_pool(name="psB", bufs=1, space="PSUM") as psB:
                wh1row = consts.tile([1, ROWS], F16)
                for h2 in range(2):
                    pw1 = psB.tile([1, 512], F32, name="pw1", tag="pw1",
                                   bufs=2)
                    for d4 in range(4):
                        nc.tensor.matmul(
                            pw1[:], v1_t[d4],
                            ndown_t[d4][:, h2 * 512:(h2 + 1) * 512],
                            start=(d4 == 0), stop=(d4 == 3),
                            skip_group_check=True)
                    nc.vector.tensor_scalar(
                        wh1row[:, h2 * 512:(h2 + 1) * 512], pw1[:], c1[:],
                        None, op0=ADD)
                p_row = consts.tile([1, ROWS], F16)
                nc.scalar.activation(p_row[:], wh1row[:], EXP, scale=ALPHA * 4)
                p_b = consts.tile([128, ROWS], F16)
                nc.gpsimd.partition_broadcast(p_b[:], p_row[:])

                for b in range(8):
                    ndT = ndpool.tile([128, 4, ROWS], F16, name="ndT",
                                      tag="ndT")
                    for d4 in range(4):
                        nc.sync.dma_start(
                            ndT[:, d4, :],
                            nodesT_d[d4 * 128:(d4 + 1) * 128,
                                     b * 1024:(b + 1) * 1024])
                    for ckl in range(8):
                        ck = b * 8 + ckl
                        pwh = psB.tile([128, 258], F32, name="pwh", tag="pwh",
                                       bufs=4)
                        for d4 in range(4):
                            nc.tensor.matmul(
                                pwh[:],
                                ndT[:, d4, ckl * 128:(ckl + 1) * 128],
                                wt_t[d4][:],
                                start=(d4 == 0), stop=(d4 == 3),
                                skip_group_check=True)
                        # one full-width drain per chunk, split across engines
                        if ck % 2 == 0:
                            nc.vector.tensor_copy(wh_aug[:, ck, :], pwh[:])
                        else:
                            nc.scalar.copy(wh_aug[:, ck, :], pwh[:])
                        if ckl == 7:
                            # q/r exps read Wh2 (fp16) strided from wh_aug
                            sl = slice(b * 8, (b + 1) * 8)
                            wh2c = wh_aug[:, sl, 257:258]
                            nc.scalar.activation(q128[:, sl], wh2c,
                                                 EXP, scale=ALPHA,
                                                 bias=c2qr[:, 0:1])
                            nc.scalar.activation(r128[:, sl], wh2c,
                                                 EXP, scale=1.0,
                                                 bias=c2qr[:, 1:2])
                nc.gpsimd.memset(wh_aug[:, :, 256:257], 1.0)

            # ---- phase 2: masked-score sweep, 8 full-PSUM accumulators ----
            with tc.tile_pool(name="psA", bufs=1, space="PSUM") as psA:
                accs = [psA.tile([128, 512], F32, name=f"acc{ib}",
                                 tag=f"acc{ib}") for ib in range(8)]
                for g in range(NG):
                    mgrp = mgp.tile([128, GRP, ROWS], F16, name="mgrp",
                                    tag="mgrp")
                    nc.gpsimd.dma_start(mgrp[:], maskm_d[g],
                                        max_dma_last_dim=512)
                    wgrp = sgp.tile([128, GRP, ROWS], F16, name="wgrp",
                                    tag="wgrp", bufs=2)
                    for ckl in range(GRP):
                        ck = g * GRP + ckl
                        nc.vector.tensor_scalar(
                            wgrp[:, ckl, :], p_b[:], r128[:, ck:ck + 1],
                            q128[:, ck:ck + 1], op0=MULT, op1=MAX)
                    ygrp = xgp.tile([128, GRP, ROWS], F16, name="ygrp",
                                    tag="ygrp")
                    nc.vector.tensor_tensor(ygrp[:], wgrp[:], mgrp[:],
                                            op=MULT)
                    for ckl in range(GRP):
                        ck = g * GRP + ckl
                        for ib in range(8):
                            nc.tensor.matmul(
                                accs[ib][:, 0:257],
                                ygrp[:, ckl, ib * 128:(ib + 1) * 128],
                                wh_aug[:, ck, 0:257],
                                start=(ck == 0), stop=(ck == NCK - 1),
                                skip_group_check=True)
                for ib in range(8):
                    o = outp.tile([128, 257], F32, name="o", tag="o")
                    nc.scalar.copy(o[:], accs[ib][:, 0:257])
                    r0 = ib * 128
                    nc.sync.dma_start(out_d[r0:r0 + 128, :], o[:])
    nc.compile()
    return nc


def _get_nc():
    if "nc" not in _CACHE:
        _CACHE["nc"] = _build_nc()
    return _CACHE["nc"]


def _prep_in_maps(nodes, edge_mat, W_w, W_b, a1_w, a1_b, a2_w, a2_b):
    import ml_dtypes
    f16 = np.float16
    f8 = ml_dtypes.float8_e4m3fn
    nodes = np.asarray(nodes, dtype=np.float32)
    edge_mat = np.asarray(edge_mat, dtype=bool)
    W_w = np.asarray(W_w, dtype=np.float32)
    W_b = np.asarray(W_b, dtype=np.float32)
    a1_w = np.asarray(a1_w, dtype=np.float32)
    a1_b = np.asarray(a1_b, dtype=np.float32)
    a2_w = np.asarray(a2_w, dtype=np.float32)
    a2_b = np.asarray(a2_b, dtype=np.float32)

    nodesT = np.ascontiguousarray(nodes.T).astype(f16)          # [512, 8192]
    v1 = (W_w.T @ a1_w[0]).astype(f16)[:, None]                 # [512, 1]
    v2 = (W_w.T @ a2_w[0]).astype(f16)[:, None]
    wt_aug = np.concatenate(
        [W_w.T.astype(f16), np.zeros((IN_DIM, 1), f16), v2], axis=1)
    c1v = float(W_b @ a1_w[0]) + float(a1_b[0])
    c2v = float(W_b @ a2_w[0]) + float(a2_b[0])
    c1 = np.array([[c1v]], np.float32)
    # q = exp(ALPHA*(wh2 + c2)); r = exp(wh2 + c2)
    c2qr = np.broadcast_to(
        np.array([ALPHA * c2v, c2v], np.float32)[None, :], (128, 2)).copy()

    # mask {0,1} fp8 grouped: m[g, p, ckl*1024 + i] = edge[i_glob, (g*4+ckl)*128+p]
    em8 = np.where(edge_mat, 1, 0).astype(f8)

    in_maps = []
    for c in range(N_CORES):
        sl = slice(c * ROWS, (c + 1) * ROWS)
        sub = np.ascontiguousarray(em8[sl, :].T)               # [8192 j, 1024 i]
        maskm = np.ascontiguousarray(
            sub.reshape(NG, GRP, 128, ROWS).transpose(0, 2, 1, 3)
        ).reshape(NG, 128, GRP * ROWS)
        in_maps.append({
            "nodesT": nodesT,
            "nodesT_own": np.ascontiguousarray(nodesT[:, sl]),
            "maskm": maskm,
            "wt_aug": wt_aug,
            "v1": v1,
            "c1": c1,
            "c2qr": c2qr,
        })
    return in_maps


def _run(inputs, trace=False, trace_cores=None):
    from concourse.bass_utils import run_bass_kernel_spmd
    if trace:
        _ensure_ntff_hook()
    nc = _get_nc()
    in_maps = _prep_in_maps(**inputs)
    res = run_bass_kernel_spmd(nc, in_maps, list(range(N_CORES)),
                               trace=trace, trace_cores=trace_cores)
    raw = np.concatenate([res.results[c]["out"] for c in range(N_CORES)],
                         axis=0)                               # [N, 257]
    W_b = np.asarray(inputs["W_b"], dtype=np.float32)
    out = raw[:, :OUT_DIM] / raw[:, OUT_DIM:OUT_DIM + 1] + W_b[None, :]
    return out.astype(np.float32), res


def kernel(**inputs) -> np.ndarray:
    out, _ = _run(inputs, trace=False)
    return out
